# revision 9
# baseline (speedup 1.0000x reference)
"""CAM (channel attention) module kernel for Trainium2, 8-core data-parallel.

Reference computation (per sample, C=512, HW=4096):
    v = x.reshape(C, HW)
    E = v @ v.T                                  # (C, C)
    att = softmax(rowmax(E) - E, axis=-1)        # == softmax(-E) stabilized at rowmin
    o = att @ v                                  # (C, HW)
    o = softmax(o, axis=-1)
    out = x + gamma * o
Sharding: data-parallel over batch B=16 -> 2 samples per NeuronCore.

v4 notes:
- x loaded via SWDGE cast-DMA (fp32 HBM -> bf16 SBUF); output stored bf16
  (host upcasts), halving HBM store traffic.  out == bf16(x) for gamma=0.
- All four E row-blocks accumulate concurrently inside the transpose
  t-loop (4 E PSUM tiles + [P,512] mm2 chunks fit the 8 banks), so mm1
  finishes with the loads instead of needing a second pass.
- Sample 0's fp8 natural-v casts run as column chunks on ACT/DVE inside
  the load window; sample 1's ride SWDGE cast-DMAs on the post-load idle
  DMA engines.
- Sample 1's mm1 + transposes interleave with sample 0's mm2 phase on
  the PE; its evictions go to ACT to keep DVE clear for the tail
  x + (gamma/Z2)*exp chunks.
"""

import sys

if "/opt/trn_rl_repo" not in sys.path:
    sys.path.insert(0, "/opt/trn_rl_repo")

from contextlib import ExitStack

import numpy as np

P = 128
C = 512
HW = 4096
HH = HW // 2
S = 2  # samples per core
CB = C // P  # 4 channel blocks
NB = HW // P  # 32 spatial blocks
NT = NB // 2  # 16 DoubleRow k-pairs for matmul 1
NJ = HW // 1024  # 4 psum chunks for the second matmul
N_CORES = 8

_NC = None


def _build_nc():
    import concourse.bacc as bacc
    import concourse.mybir as mybir
    import concourse.tile as tile
    from concourse.masks import make_identity

    f32 = mybir.dt.float32
    bf16 = mybir.dt.bfloat16
    fp8 = mybir.dt.float8e4
    AF = mybir.ActivationFunctionType
    ALU = mybir.AluOpType
    AX = mybir.AxisListType
    DR = mybir.MatmulPerfMode.DoubleRow

    nc = bacc.Bacc(
        "TRN2",
        target_bir_lowering=False,
        debug=False,
        num_devices=N_CORES,
        num_swdge_queues=4,
    )
    x = nc.dram_tensor("x", (S, C, HW), f32, kind="ExternalInput").ap()
    gamma = nc.dram_tensor("gamma", (1,), f32, kind="ExternalInput").ap()
    out = nc.dram_tensor("out", (S, C, HW), bf16, kind="ExternalOutput").ap()

    with tile.TileContext(nc) as tc, ExitStack() as ctx:
        const = ctx.enter_context(tc.tile_pool(name="const", bufs=1))
        identb = const.tile([P, P], bf16)
        make_identity(nc, identb)
        gamma_sb = const.tile([P, 1], f32)
        nc.sync.dma_start(out=gamma_sb, in_=gamma.to_broadcast((P, 1)))

        xf_pool = ctx.enter_context(tc.tile_pool(name="xf_pool", bufs=8))
        vb_pool = ctx.enter_context(tc.tile_pool(name="vb_pool", bufs=3))
        vt_pool = ctx.enter_context(tc.tile_pool(name="vt_pool", bufs=NT + 2))
        att_pool = ctx.enter_context(tc.tile_pool(name="att_pool", bufs=CB + 1))
        attT_pool = ctx.enter_context(tc.tile_pool(name="attT_pool", bufs=3))
        exp_pool = ctx.enter_context(tc.tile_pool(name="exp_pool", bufs=3))
        small = ctx.enter_context(tc.tile_pool(name="small", bufs=12))
        r1_pool = ctx.enter_context(tc.tile_pool(name="r1_pool", bufs=10))
        psum_E = ctx.enter_context(tc.tile_pool(name="psum_E", bufs=4, space="PSUM"))
        psum_mix = ctx.enter_context(
            tc.tile_pool(name="psum_mix", bufs=2, space="PSUM")
        )

        # per-sample state
        xh = [[None] * CB for _ in range(S)]
        vb2 = [[None] * (CB // 2) for _ in range(S)]
        vT2 = [[None] * NT for _ in range(S)]
        att8 = [[None] * CB for _ in range(S)]
        r1s = [[None] * CB for _ in range(S)]
        attT2 = [[None] * (CB // 2) for _ in range(S)]
        Es = [[None] * CB for _ in range(S)]

        def loads(s, chunks):
            # SWDGE cast-DMA loads: fp32 HBM -> bf16 SBUF, column-chunked so
            # the first transposes start as soon as the first chunk lands.
            for i in range(CB):
                xh[s][i] = xf_pool.tile([P, HW], bf16, tag="xf", name=f"xf_{s}_{i}")
            c0 = 0
            for w in chunks:
                for i in range(CB):
                    nc.gpsimd.dma_start(
                        out=xh[s][i][:, c0 : c0 + w],
                        in_=x[s, i * P : (i + 1) * P, c0 : c0 + w],
                    )
                c0 += w
            assert c0 == HW

        def vb_alloc(s):
            for u in range(CB // 2):
                vb2[s][u] = vb_pool.tile(
                    [P, 2, HW], fp8, tag="vb", name=f"vb2_{s}_{u}"
                )

        def vb_cast_chunk_eng(s, q):
            # bf16 -> fp8 natural-layout copies, one 1024-col chunk, ACT/DVE.
            sl = slice(q * 1024, (q + 1) * 1024)
            for u in range(CB // 2):
                for ko in range(2):
                    i = 2 * u + ko
                    if i % 2 == 0:
                        nc.scalar.copy(vb2[s][u][:, ko, sl], xh[s][i][:, sl])
                    else:
                        nc.vector.tensor_copy(vb2[s][u][:, ko, sl], xh[s][i][:, sl])

        def vb_casts_dma(s):
            # bf16 -> fp8 SWDGE cast-DMAs, half-width; drain post-load on
            # otherwise-idle DMA engines.
            for h in range(2):
                sl = slice(h * HH, (h + 1) * HH)
                for u in range(CB // 2):
                    for ko in range(2):
                        i = 2 * u + ko
                        nc.gpsimd.dma_start(
                            out=vb2[s][u][:, ko, sl], in_=xh[s][i][:, sl]
                        )

        def v_transposes(s, ts, te, evict_eng=None):
            # vT pairs (n-part, c-free) fp8: bf16 PE transposes (PSUM dtype
            # matches input), cast to fp8 during the PSUM->SBUF eviction.
            for t in range(ts, te):
                vt_ = vt_pool.tile([P, 2, C], fp8, tag="vt", name=f"vT2_{s}_{t}")
                for ko in range(2):
                    k = 2 * t + ko
                    pt = psum_mix.tile([P, C], bf16, tag="mix", name=f"ptv_{s}_{k}")
                    for i in range(CB):
                        nc.tensor.transpose(
                            pt[:, i * P : (i + 1) * P],
                            xh[s][i][:, k * P : (k + 1) * P],
                            identb,
                        )
                    eng = evict_eng if evict_eng else ("v" if k % 2 else "s")
                    if eng == "v":
                        nc.vector.tensor_copy(vt_[:, ko, :], pt)
                    else:
                        nc.scalar.copy(vt_[:, ko, :], pt)
                vT2[s][t] = vt_

        def softmax1_tail(s, i):
            E = Es[s][i]
            m = small.tile([P, 1], f32, tag="sm", name=f"m_{s}_{i}")
            nc.vector.tensor_reduce(m, E, axis=AX.X, op=ALU.min)
            a = att_pool.tile([P, C], bf16, tag="att", name=f"att_{s}_{i}")
            z1 = small.tile([P, 1], f32, tag="sm", name=f"z1_{s}_{i}")
            nc.scalar.activation(a, E, AF.Exp, bias=m, scale=-1.0, accum_out=z1)
            r1 = r1_pool.tile([P, 1], f32, tag="r1", name=f"r1_{s}_{i}")
            nc.vector.reciprocal(r1, z1)
            att8[s][i] = a
            r1s[s][i] = r1

        def mm1_alloc(s):
            Es[s] = [
                psum_E.tile([P, C], f32, tag="E", name=f"E_{s}_{i}")
                for i in range(CB)
            ]

        def mm1_blocks(s, t):
            # one DoubleRow k-pair step of E = v @ v.T for all 4 row-blocks
            for i in range(CB):
                nc.tensor.matmul(
                    Es[s][i],
                    lhsT=vT2[s][t][:, :, i * P : (i + 1) * P],
                    rhs=vT2[s][t],
                    perf_mode=DR,
                    start=(t == 0),
                    stop=(t == NT - 1),
                )

        def att_transposes(s):
            # attT pairs (col-part, row-free) fp8 via bf16 PE transpose
            for u in range(CB // 2):
                st = attT_pool.tile([P, 2, C], fp8, tag="attT", name=f"attT2_{s}_{u}")
                for ko in range(2):
                    j = 2 * u + ko
                    pt = psum_mix.tile([P, C], bf16, tag="mix", name=f"pta_{s}_{j}")
                    for i in range(CB):
                        nc.tensor.transpose(
                            pt[:, i * P : (i + 1) * P],
                            att8[s][i][:, j * P : (j + 1) * P],
                            identb,
                        )
                    if j % 2 == 0 and s == 0:
                        nc.vector.tensor_copy(st[:, ko, :], pt)
                    else:
                        nc.scalar.copy(st[:, ko, :], pt)
                attT2[s][u] = st

        def mm2_final(s, i):
            # o = att @ v (DoubleRow) in [P,512] PSUM chunks, softmax over HW
            # (1/Z1 folded into the exp scale), then out = x + (gamma/Z2)*exp
            # in half-width DVE chunks each followed by its bf16 half-store.
            er = exp_pool.tile([P, HW], bf16, tag="er", name=f"er_{s}_{i}")
            z2p = small.tile([P, NJ], f32, tag="z2p", name=f"z2p_{s}_{i}")
            for nj in range(NJ):
                o2 = psum_mix.tile(
                    [P, 1024], f32, tag="mix", name=f"o2_{s}_{i}_{nj}"
                )
                for hh in range(2):
                    sl = slice(nj * 1024 + hh * 512, nj * 1024 + (hh + 1) * 512)
                    for u in range(CB // 2):
                        nc.tensor.matmul(
                            o2[:, hh * 512 : (hh + 1) * 512],
                            lhsT=attT2[s][u][:, :, i * P : (i + 1) * P],
                            rhs=vb2[s][u][:, :, sl],
                            perf_mode=DR,
                            start=(u == 0),
                            stop=(u == CB // 2 - 1),
                        )
                nc.scalar.activation(
                    er[:, nj * 1024 : (nj + 1) * 1024],
                    o2,
                    AF.Exp,
                    scale=r1s[s][i],
                    accum_out=z2p[:, nj : nj + 1],
                )
            z2 = small.tile([P, 1], f32, tag="sm", name=f"z2_{s}_{i}")
            nc.vector.reduce_sum(z2, z2p, axis=AX.X)
            r2 = small.tile([P, 1], f32, tag="sm", name=f"r2_{s}_{i}")
            nc.vector.reciprocal(r2, z2)
            gz = small.tile([P, 1], f32, tag="sm", name=f"gz_{s}_{i}")
            nc.vector.tensor_scalar_mul(gz, r2, gamma_sb)
            xt = xh[s][i]
            for h in range(2):
                sl = slice(h * HH, (h + 1) * HH)
                nc.vector.scalar_tensor_tensor(
                    out=xt[:, sl],
                    in0=er[:, sl],
                    scalar=gz,
                    in1=xt[:, sl],
                    op0=ALU.mult,
                    op1=ALU.add,
                )
                nc.sync.dma_start(
                    out=out[s, i * P : (i + 1) * P, sl],
                    in_=xt[:, sl],
                )

        # ---- software pipeline across the two samples ----
        loads(0, [512, 512, 1024, 1024, 1024])
        loads(1, [2048, 2048])
        vb_alloc(0)
        mm1_alloc(0)
        for t in range(NT):
            v_transposes(0, t, t + 1)
            mm1_blocks(0, t)
            if t % 4 == 3:
                # fp8 natural-v cast chunk for the columns just consumed
                vb_cast_chunk_eng(0, t // 4)
        for i in range(CB):
            softmax1_tail(0, i)
        att_transposes(0)
        vb_alloc(1)
        vb_casts_dma(1)
        mm1_alloc(1)
        for i in range(CB):
            mm2_final(0, i)
            v_transposes(1, i * (NT // CB), (i + 1) * (NT // CB))
            for t in range(i * (NT // CB), (i + 1) * (NT // CB)):
                mm1_blocks(1, t)
        for i in range(CB):
            softmax1_tail(1, i)
        att_transposes(1)
        for i in range(CB):
            mm2_final(1, i)

    nc.compile()
    return nc


def get_nc():
    global _NC
    if _NC is None:
        _NC = _build_nc()
    return _NC


def kernel(x: np.ndarray, gamma: np.ndarray) -> np.ndarray:
    from concourse.bass_utils import run_bass_kernel_spmd

    B, Cx, H, W = x.shape
    assert (B, Cx, H * W) == (16, C, HW), (B, Cx, H, W)
    nc = get_nc()
    xs = np.ascontiguousarray(np.asarray(x, dtype=np.float32)).reshape(B, Cx, H * W)
    g = np.ascontiguousarray(np.asarray(gamma, dtype=np.float32)).reshape(1)
    in_maps = [{"x": xs[S * c : S * (c + 1)], "gamma": g} for c in range(N_CORES)]
    res = run_bass_kernel_spmd(nc, in_maps, core_ids=list(range(N_CORES)))
    out = np.concatenate(
        [np.asarray(res.results[c]["out"]).astype(np.float32) for c in range(N_CORES)],
        axis=0,
    )
    return out.reshape(B, Cx, H, W)


# revision 10
# speedup vs baseline: 1.0768x; 1.0768x over previous
"""CAM (channel attention) module kernel for Trainium2, 8-core data-parallel.

Reference computation (per sample, C=512, HW=4096):
    v = x.reshape(C, HW)
    E = v @ v.T                                  # (C, C)
    att = softmax(rowmax(E) - E, axis=-1)        # == softmax(-E) stabilized at rowmin
    o = att @ v                                  # (C, HW)
    o = softmax(o, axis=-1)
    out = x + gamma * o
Sharding: data-parallel over batch B=16 -> 2 samples per NeuronCore.

v6 notes:
- x loaded via SWDGE cast-DMA (fp32 HBM -> bf16 SBUF); output stored bf16
  (host upcasts), halving HBM store traffic.  out == bf16(x) for gamma=0.
- PSUM (8 banks): E-pair pool (2) + transpose-pair pool (2) + two
  [P,1024] mm2 chunks (4).  Sample 0's mm1 still runs all 4 row-blocks
  concurrently through the load window by borrowing the then-idle mm2
  chunk slots for E rows 2,3; sample 1 accumulates rows 0,1 during
  sample 0's mm2 phase and rows 2,3 in one dense pass after it.
- Sample 1's transposes are interleaved between mm2 PSUM chunks so the
  PE fills its exp-drain waits; its mm1 blocks are emitted one iteration
  later so eviction latency never stalls the PE queue.
- fp8 natural-layout v: sample 0 via ACT/DVE column-chunk copies inside
  the load window, sample 1 via SWDGE cast-DMAs on post-load idle DMA.
- Softmax1 normalization is folded into the second exp's scale; final
  out = x + (gamma/Z2)*exp runs as half-width DVE chunks, each followed
  by its bf16 half-store.
"""

import sys

if "/opt/trn_rl_repo" not in sys.path:
    sys.path.insert(0, "/opt/trn_rl_repo")

from contextlib import ExitStack

import numpy as np

P = 128
C = 512
HW = 4096
HH = HW // 2
S = 2  # samples per core
CB = C // P  # 4 channel blocks
NB = HW // P  # 32 spatial blocks
NT = NB // 2  # 16 DoubleRow k-pairs for matmul 1
NJ = HW // 1024  # 4 psum chunks for the second matmul
N_CORES = 8

_NC = None


def _build_nc():
    import concourse.bacc as bacc
    import concourse.mybir as mybir
    import concourse.tile as tile
    from concourse.masks import make_identity

    f32 = mybir.dt.float32
    bf16 = mybir.dt.bfloat16
    fp8 = mybir.dt.float8e4
    AF = mybir.ActivationFunctionType
    ALU = mybir.AluOpType
    AX = mybir.AxisListType
    DR = mybir.MatmulPerfMode.DoubleRow

    nc = bacc.Bacc(
        "TRN2",
        target_bir_lowering=False,
        debug=False,
        num_devices=N_CORES,
        num_swdge_queues=4,
    )
    x = nc.dram_tensor("x", (S, C, HW), f32, kind="ExternalInput").ap()
    gamma = nc.dram_tensor("gamma", (1,), f32, kind="ExternalInput").ap()
    out = nc.dram_tensor("out", (S, C, HW), bf16, kind="ExternalOutput").ap()

    with tile.TileContext(nc) as tc, ExitStack() as ctx:
        const = ctx.enter_context(tc.tile_pool(name="const", bufs=1))
        identb = const.tile([P, P], bf16)
        make_identity(nc, identb)
        gamma_sb = const.tile([P, 1], f32)
        nc.sync.dma_start(out=gamma_sb, in_=gamma.to_broadcast((P, 1)))

        xf_pool = ctx.enter_context(tc.tile_pool(name="xf_pool", bufs=8))
        vb_pool = ctx.enter_context(tc.tile_pool(name="vb_pool", bufs=3))
        vt_pool = ctx.enter_context(tc.tile_pool(name="vt_pool", bufs=NT + 2))
        att_pool = ctx.enter_context(tc.tile_pool(name="att_pool", bufs=CB + 1))
        attT_pool = ctx.enter_context(tc.tile_pool(name="attT_pool", bufs=3))
        exp_pool = ctx.enter_context(tc.tile_pool(name="exp_pool", bufs=3))
        small = ctx.enter_context(tc.tile_pool(name="small", bufs=12))
        r1_pool = ctx.enter_context(tc.tile_pool(name="r1_pool", bufs=10))
        psum_E = ctx.enter_context(tc.tile_pool(name="psum_E", bufs=2, space="PSUM"))
        psum_pt = ctx.enter_context(tc.tile_pool(name="psum_pt", bufs=2, space="PSUM"))
        psum_o = ctx.enter_context(tc.tile_pool(name="psum_o", bufs=2, space="PSUM"))

        # per-sample state
        xh = [[None] * CB for _ in range(S)]
        vb2 = [[None] * (CB // 2) for _ in range(S)]
        vT2 = [[None] * NT for _ in range(S)]
        att8 = [[None] * CB for _ in range(S)]
        r1s = [[None] * CB for _ in range(S)]
        attT2 = [[None] * (CB // 2) for _ in range(S)]
        Es = [[None] * CB for _ in range(S)]

        def loads(s, chunks):
            # SWDGE cast-DMA loads: fp32 HBM -> bf16 SBUF, column-chunked so
            # the first transposes start as soon as the first chunk lands.
            for i in range(CB):
                xh[s][i] = xf_pool.tile([P, HW], bf16, tag="xf", name=f"xf_{s}_{i}")
            c0 = 0
            for w in chunks:
                for i in range(CB):
                    nc.gpsimd.dma_start(
                        out=xh[s][i][:, c0 : c0 + w],
                        in_=x[s, i * P : (i + 1) * P, c0 : c0 + w],
                    )
                c0 += w
            assert c0 == HW

        def vb_alloc(s):
            for u in range(CB // 2):
                vb2[s][u] = vb_pool.tile(
                    [P, 2, HW], fp8, tag="vb", name=f"vb2_{s}_{u}"
                )

        def vb_cast_chunk_eng(s, q):
            # bf16 -> fp8 natural-layout copies, one 1024-col chunk, ACT/DVE.
            sl = slice(q * 1024, (q + 1) * 1024)
            for u in range(CB // 2):
                for ko in range(2):
                    i = 2 * u + ko
                    if i % 2 == 0:
                        nc.scalar.copy(vb2[s][u][:, ko, sl], xh[s][i][:, sl])
                    else:
                        nc.vector.tensor_copy(vb2[s][u][:, ko, sl], xh[s][i][:, sl])

        def vb_casts_dma(s):
            # bf16 -> fp8 SWDGE cast-DMAs, half-width; drain post-load on
            # otherwise-idle DMA engines.
            for h in range(2):
                sl = slice(h * HH, (h + 1) * HH)
                for u in range(CB // 2):
                    for ko in range(2):
                        i = 2 * u + ko
                        nc.gpsimd.dma_start(
                            out=vb2[s][u][:, ko, sl], in_=xh[s][i][:, sl]
                        )

        def v_transposes(s, ts, te):
            # vT pairs (n-part, c-free) fp8: bf16 PE transposes (PSUM dtype
            # matches input), cast to fp8 during the PSUM->SBUF eviction.
            for t in range(ts, te):
                vt_ = vt_pool.tile([P, 2, C], fp8, tag="vt", name=f"vT2_{s}_{t}")
                for ko in range(2):
                    k = 2 * t + ko
                    pt = psum_pt.tile([P, C], bf16, tag="pt", name=f"ptv_{s}_{k}")
                    for i in range(CB):
                        nc.tensor.transpose(
                            pt[:, i * P : (i + 1) * P],
                            xh[s][i][:, k * P : (k + 1) * P],
                            identb,
                        )
                    if k % 2 == 1:
                        nc.vector.tensor_copy(vt_[:, ko, :], pt)
                    else:
                        nc.scalar.copy(vt_[:, ko, :], pt)
                vT2[s][t] = vt_

        def softmax1_tail(s, i):
            E = Es[s][i]
            m = small.tile([P, 1], f32, tag="sm", name=f"m_{s}_{i}")
            nc.vector.tensor_reduce(m, E, axis=AX.X, op=ALU.min)
            a = att_pool.tile([P, C], bf16, tag="att", name=f"att_{s}_{i}")
            z1 = small.tile([P, 1], f32, tag="sm", name=f"z1_{s}_{i}")
            nc.scalar.activation(a, E, AF.Exp, bias=m, scale=-1.0, accum_out=z1)
            r1 = r1_pool.tile([P, 1], f32, tag="r1", name=f"r1_{s}_{i}")
            nc.vector.reciprocal(r1, z1)
            att8[s][i] = a
            r1s[s][i] = r1

        def mm1_block(s, i, t):
            nc.tensor.matmul(
                Es[s][i],
                lhsT=vT2[s][t][:, :, i * P : (i + 1) * P],
                rhs=vT2[s][t],
                perf_mode=DR,
                start=(t == 0),
                stop=(t == NT - 1),
            )

        def att_transposes(s):
            # attT pairs (col-part, row-free) fp8 via bf16 PE transpose
            for u in range(CB // 2):
                st = attT_pool.tile([P, 2, C], fp8, tag="attT", name=f"attT2_{s}_{u}")
                for ko in range(2):
                    j = 2 * u + ko
                    pt = psum_pt.tile([P, C], bf16, tag="pt", name=f"pta_{s}_{j}")
                    for i in range(CB):
                        nc.tensor.transpose(
                            pt[:, i * P : (i + 1) * P],
                            att8[s][i][:, j * P : (j + 1) * P],
                            identb,
                        )
                    if j % 2 == 0 and s == 0:
                        nc.vector.tensor_copy(st[:, ko, :], pt)
                    else:
                        nc.scalar.copy(st[:, ko, :], pt)
                attT2[s][u] = st

        def mm2_final(s, i, interleave_ts=None):
            # o = att @ v (DoubleRow) in [P,1024] PSUM chunks, softmax over
            # HW (1/Z1 folded into the exp scale).  interleave_ts: sample-1
            # transpose steps slotted between chunks so the PE rides out the
            # exp drain of each chunk.  Then out = x + (gamma/Z2)*exp in
            # half-width DVE chunks, each followed by its bf16 half-store.
            er = exp_pool.tile([P, HW], bf16, tag="er", name=f"er_{s}_{i}")
            z2p = small.tile([P, NJ], f32, tag="z2p", name=f"z2p_{s}_{i}")
            for nj in range(NJ):
                o2 = psum_o.tile([P, 1024], f32, tag="o", name=f"o2_{s}_{i}_{nj}")
                for hh in range(2):
                    sl = slice(nj * 1024 + hh * 512, nj * 1024 + (hh + 1) * 512)
                    for u in range(CB // 2):
                        nc.tensor.matmul(
                            o2[:, hh * 512 : (hh + 1) * 512],
                            lhsT=attT2[s][u][:, :, i * P : (i + 1) * P],
                            rhs=vb2[s][u][:, :, sl],
                            perf_mode=DR,
                            start=(u == 0),
                            stop=(u == CB // 2 - 1),
                        )
                if interleave_ts is not None:
                    v_transposes(1, interleave_ts[nj], interleave_ts[nj] + 1)
                nc.scalar.activation(
                    er[:, nj * 1024 : (nj + 1) * 1024],
                    o2,
                    AF.Exp,
                    scale=r1s[s][i],
                    accum_out=z2p[:, nj : nj + 1],
                )
            z2 = small.tile([P, 1], f32, tag="sm", name=f"z2_{s}_{i}")
            nc.vector.reduce_sum(z2, z2p, axis=AX.X)
            r2 = small.tile([P, 1], f32, tag="sm", name=f"r2_{s}_{i}")
            nc.vector.reciprocal(r2, z2)
            gz = small.tile([P, 1], f32, tag="sm", name=f"gz_{s}_{i}")
            nc.vector.tensor_scalar_mul(gz, r2, gamma_sb)
            xt = xh[s][i]
            for h in range(2):
                sl = slice(h * HH, (h + 1) * HH)
                nc.vector.scalar_tensor_tensor(
                    out=xt[:, sl],
                    in0=er[:, sl],
                    scalar=gz,
                    in1=xt[:, sl],
                    op0=ALU.mult,
                    op1=ALU.add,
                )
                nc.sync.dma_start(
                    out=out[s, i * P : (i + 1) * P, sl],
                    in_=xt[:, sl],
                )

        # ---- software pipeline across the two samples ----
        loads(0, [512, 512, 1024, 1024, 1024])
        loads(1, [2048, 2048])
        vb_alloc(0)
        # sample-0 E quad: rows 0,1 in the E pool; rows 2,3 borrow the mm2
        # chunk slots (idle until sample-0's mm2 phase begins).
        Es[0] = [
            psum_E.tile([P, C], f32, tag="E", name="E_0_0"),
            psum_E.tile([P, C], f32, tag="E", name="E_0_1"),
            psum_o.tile([P, C], f32, tag="o", name="E_0_2"),
            psum_o.tile([P, C], f32, tag="o", name="E_0_3"),
        ]
        for t in range(NT):
            v_transposes(0, t, t + 1)
            for i in range(CB):
                mm1_block(0, i, t)
            if t in (3, 7, 11):
                vb_cast_chunk_eng(0, t // 4)
        for i in range(CB):
            softmax1_tail(0, i)
        vb_cast_chunk_eng(0, 3)
        att_transposes(0)
        vb_alloc(1)
        vb_casts_dma(1)
        # sample-1 rows 0,1 accumulate through sample-0's mm2 phase
        Es[1][0] = psum_E.tile([P, C], f32, tag="E", name="E_1_0")
        Es[1][1] = psum_E.tile([P, C], f32, tag="E", name="E_1_1")
        prev_ts = []
        for i in range(CB):
            for t in prev_ts:
                mm1_block(1, 0, t)
                mm1_block(1, 1, t)
            prev_ts = list(range(i * NJ, (i + 1) * NJ))
            mm2_final(0, i, interleave_ts=prev_ts)
        for t in prev_ts:
            mm1_block(1, 0, t)
            mm1_block(1, 1, t)
        softmax1_tail(1, 0)
        softmax1_tail(1, 1)
        # sample-1 rows 2,3: one dense pass over the vT tiles
        Es[1][2] = psum_E.tile([P, C], f32, tag="E", name="E_1_2")
        Es[1][3] = psum_E.tile([P, C], f32, tag="E", name="E_1_3")
        for t in range(NT):
            mm1_block(1, 2, t)
            mm1_block(1, 3, t)
        softmax1_tail(1, 2)
        softmax1_tail(1, 3)
        att_transposes(1)
        for i in range(CB):
            mm2_final(1, i)

    nc.compile()
    return nc


def get_nc():
    global _NC
    if _NC is None:
        _NC = _build_nc()
    return _NC


def kernel(x: np.ndarray, gamma: np.ndarray) -> np.ndarray:
    from concourse.bass_utils import run_bass_kernel_spmd

    B, Cx, H, W = x.shape
    assert (B, Cx, H * W) == (16, C, HW), (B, Cx, H, W)
    nc = get_nc()
    xs = np.ascontiguousarray(np.asarray(x, dtype=np.float32)).reshape(B, Cx, H * W)
    g = np.ascontiguousarray(np.asarray(gamma, dtype=np.float32)).reshape(1)
    in_maps = [{"x": xs[S * c : S * (c + 1)], "gamma": g} for c in range(N_CORES)]
    res = run_bass_kernel_spmd(nc, in_maps, core_ids=list(range(N_CORES)))
    out = np.concatenate(
        [np.asarray(res.results[c]["out"]).astype(np.float32) for c in range(N_CORES)],
        axis=0,
    )
    return out.reshape(B, Cx, H, W)


# revision 13
# speedup vs baseline: 1.0985x; 1.0202x over previous
"""CAM (channel attention) module kernel for Trainium2, 8-core data-parallel.

Reference computation (per sample, C=512, HW=4096):
    v = x.reshape(C, HW)
    E = v @ v.T                                  # (C, C)
    att = softmax(rowmax(E) - E, axis=-1)        # == softmax(-E) stabilized at rowmin
    o = att @ v                                  # (C, HW)
    o = softmax(o, axis=-1)
    out = x + gamma * o
Sharding: data-parallel over batch B=16 -> 2 samples per NeuronCore.

v6 notes:
- x loaded via SWDGE cast-DMA (fp32 HBM -> bf16 SBUF); output stored bf16
  (host upcasts), halving HBM store traffic.  out == bf16(x) for gamma=0.
- PSUM (8 banks): E-pair pool (2) + transpose-pair pool (2) + two
  [P,1024] mm2 chunks (4).  Sample 0's mm1 still runs all 4 row-blocks
  concurrently through the load window by borrowing the then-idle mm2
  chunk slots for E rows 2,3; sample 1 accumulates rows 0,1 during
  sample 0's mm2 phase and rows 2,3 in one dense pass after it.
- Sample 1's transposes are interleaved between mm2 PSUM chunks so the
  PE fills its exp-drain waits; its mm1 blocks are emitted one iteration
  later so eviction latency never stalls the PE queue.
- fp8 natural-layout v: sample 0 via ACT/DVE column-chunk copies inside
  the load window, sample 1 via SWDGE cast-DMAs on post-load idle DMA.
- Softmax1 normalization is folded into the second exp's scale; final
  out = x + (gamma/Z2)*exp runs as half-width DVE chunks, each followed
  by its bf16 half-store.
"""

import sys

if "/opt/trn_rl_repo" not in sys.path:
    sys.path.insert(0, "/opt/trn_rl_repo")

from contextlib import ExitStack

import numpy as np

P = 128
C = 512
HW = 4096
HH = HW // 2
S = 2  # samples per core
CB = C // P  # 4 channel blocks
NB = HW // P  # 32 spatial blocks
NT = NB // 2  # 16 DoubleRow k-pairs for matmul 1
NJ = HW // 1024  # 4 psum chunks for the second matmul
N_CORES = 8

_NC = None


def _build_nc():
    import concourse.bacc as bacc
    import concourse.mybir as mybir
    import concourse.tile as tile
    from concourse.masks import make_identity

    f32 = mybir.dt.float32
    bf16 = mybir.dt.bfloat16
    fp8 = mybir.dt.float8e4
    AF = mybir.ActivationFunctionType
    ALU = mybir.AluOpType
    AX = mybir.AxisListType
    DR = mybir.MatmulPerfMode.DoubleRow

    nc = bacc.Bacc(
        "TRN2",
        target_bir_lowering=False,
        debug=False,
        num_devices=N_CORES,
        num_swdge_queues=4,
    )
    x = nc.dram_tensor("x", (S, C, HW), f32, kind="ExternalInput").ap()
    gamma = nc.dram_tensor("gamma", (1,), f32, kind="ExternalInput").ap()
    out = nc.dram_tensor("out", (S, C, HW), bf16, kind="ExternalOutput").ap()

    with tile.TileContext(nc) as tc, ExitStack() as ctx:
        const = ctx.enter_context(tc.tile_pool(name="const", bufs=1))
        identb = const.tile([P, P], bf16)
        make_identity(nc, identb)
        gamma_sb = const.tile([P, 1], f32)
        nc.sync.dma_start(out=gamma_sb, in_=gamma.to_broadcast((P, 1)))

        xf_pool = ctx.enter_context(tc.tile_pool(name="xf_pool", bufs=8))
        vb_pool = ctx.enter_context(tc.tile_pool(name="vb_pool", bufs=3))
        vt_pool = ctx.enter_context(tc.tile_pool(name="vt_pool", bufs=NT + 2))
        att_pool = ctx.enter_context(tc.tile_pool(name="att_pool", bufs=CB + 1))
        attT_pool = ctx.enter_context(tc.tile_pool(name="attT_pool", bufs=3))
        exp_pool = ctx.enter_context(tc.tile_pool(name="exp_pool", bufs=3))
        small = ctx.enter_context(tc.tile_pool(name="small", bufs=12))
        r1_pool = ctx.enter_context(tc.tile_pool(name="r1_pool", bufs=10))
        psum_E = ctx.enter_context(tc.tile_pool(name="psum_E", bufs=2, space="PSUM"))
        psum_pt = ctx.enter_context(tc.tile_pool(name="psum_pt", bufs=2, space="PSUM"))
        psum_o = ctx.enter_context(tc.tile_pool(name="psum_o", bufs=2, space="PSUM"))

        # per-sample state
        xh = [[None] * CB for _ in range(S)]
        vb2 = [[None] * (CB // 2) for _ in range(S)]
        vT2 = [[None] * NT for _ in range(S)]
        att8 = [[None] * CB for _ in range(S)]
        r1s = [[None] * CB for _ in range(S)]
        attT2 = [[None] * (CB // 2) for _ in range(S)]
        Es = [[None] * CB for _ in range(S)]

        def loads(s, chunks):
            # SWDGE cast-DMA loads: fp32 HBM -> bf16 SBUF, column-chunked so
            # the first transposes start as soon as the first chunk lands.
            for i in range(CB):
                xh[s][i] = xf_pool.tile([P, HW], bf16, tag="xf", name=f"xf_{s}_{i}")
            c0 = 0
            for w in chunks:
                for i in range(CB):
                    nc.gpsimd.dma_start(
                        out=xh[s][i][:, c0 : c0 + w],
                        in_=x[s, i * P : (i + 1) * P, c0 : c0 + w],
                    )
                c0 += w
            assert c0 == HW

        def vb_alloc(s):
            for u in range(CB // 2):
                vb2[s][u] = vb_pool.tile(
                    [P, 2, HW], fp8, tag="vb", name=f"vb2_{s}_{u}"
                )

        def vb_cast_chunk_eng(s, q):
            # bf16 -> fp8 natural-layout copies, one 1024-col chunk, ACT/DVE.
            sl = slice(q * 1024, (q + 1) * 1024)
            for u in range(CB // 2):
                for ko in range(2):
                    i = 2 * u + ko
                    if i % 2 == 0:
                        nc.scalar.copy(vb2[s][u][:, ko, sl], xh[s][i][:, sl])
                    else:
                        nc.vector.tensor_copy(vb2[s][u][:, ko, sl], xh[s][i][:, sl])

        def vb_casts_dma(s):
            # bf16 -> fp8 SWDGE cast-DMAs, half-width; drain post-load on
            # otherwise-idle DMA engines.
            for h in range(2):
                sl = slice(h * HH, (h + 1) * HH)
                for u in range(CB // 2):
                    for ko in range(2):
                        i = 2 * u + ko
                        nc.gpsimd.dma_start(
                            out=vb2[s][u][:, ko, sl], in_=xh[s][i][:, sl]
                        )

        def v_transposes(s, ts, te, evict_eng=None):
            # vT pairs (n-part, c-free) fp8: bf16 PE transposes (PSUM dtype
            # matches input), cast to fp8 during the PSUM->SBUF eviction.
            for t in range(ts, te):
                vt_ = vt_pool.tile([P, 2, C], fp8, tag="vt", name=f"vT2_{s}_{t}")
                for ko in range(2):
                    k = 2 * t + ko
                    pt = psum_pt.tile([P, C], bf16, tag="pt", name=f"ptv_{s}_{k}")
                    for i in range(CB):
                        nc.tensor.transpose(
                            pt[:, i * P : (i + 1) * P],
                            xh[s][i][:, k * P : (k + 1) * P],
                            identb,
                        )
                    eng = evict_eng if evict_eng else ("v" if k % 2 else "s")
                    if eng == "v":
                        nc.vector.tensor_copy(vt_[:, ko, :], pt)
                    else:
                        nc.scalar.copy(vt_[:, ko, :], pt)
                vT2[s][t] = vt_

        def softmax1_tail(s, i):
            E = Es[s][i]
            m = small.tile([P, 1], f32, tag="sm", name=f"m_{s}_{i}")
            nc.vector.tensor_reduce(m, E, axis=AX.X, op=ALU.min)
            a = att_pool.tile([P, C], bf16, tag="att", name=f"att_{s}_{i}")
            z1 = small.tile([P, 1], f32, tag="sm", name=f"z1_{s}_{i}")
            nc.scalar.activation(a, E, AF.Exp, bias=m, scale=-1.0, accum_out=z1)
            r1 = r1_pool.tile([P, 1], f32, tag="r1", name=f"r1_{s}_{i}")
            nc.vector.reciprocal(r1, z1)
            att8[s][i] = a
            r1s[s][i] = r1

        def pe_warm(n):
            # dependency-free LDWEIGHTS keep the PE HAM activity monitor
            # from re-throttling the clock across data-wait gaps; they run
            # the instant the PE reaches them and are overwritten by the
            # next matmul's own weight load.
            for _ in range(n):
                nc.tensor.ldweights(identb)

        def mm1_block(s, i, t):
            nc.tensor.matmul(
                Es[s][i],
                lhsT=vT2[s][t][:, :, i * P : (i + 1) * P],
                rhs=vT2[s][t],
                perf_mode=DR,
                start=(t == 0),
                stop=(t == NT - 1),
            )

        def att_transposes(s):
            # attT pairs (col-part, row-free) fp8 via bf16 PE transpose
            for u in range(CB // 2):
                st = attT_pool.tile([P, 2, C], fp8, tag="attT", name=f"attT2_{s}_{u}")
                for ko in range(2):
                    j = 2 * u + ko
                    pt = psum_pt.tile([P, C], bf16, tag="pt", name=f"pta_{s}_{j}")
                    for i in range(CB):
                        nc.tensor.transpose(
                            pt[:, i * P : (i + 1) * P],
                            att8[s][i][:, j * P : (j + 1) * P],
                            identb,
                        )
                    if j % 2 == 0 and s == 0:
                        nc.vector.tensor_copy(st[:, ko, :], pt)
                    else:
                        nc.scalar.copy(st[:, ko, :], pt)
                attT2[s][u] = st

        def mm2_final(s, i, interleave_ts=None):
            # o = att @ v (DoubleRow) in [P,1024] PSUM chunks, softmax over
            # HW (1/Z1 folded into the exp scale).  interleave_ts: sample-1
            # transpose steps slotted between chunks so the PE rides out the
            # exp drain of each chunk.  Then out = x + (gamma/Z2)*exp in
            # half-width DVE chunks, each followed by its bf16 half-store.
            er = exp_pool.tile([P, HW], bf16, tag="er", name=f"er_{s}_{i}")
            z2p = small.tile([P, NJ], f32, tag="z2p", name=f"z2p_{s}_{i}")
            for nj in range(NJ):
                o2 = psum_o.tile([P, 1024], f32, tag="o", name=f"o2_{s}_{i}_{nj}")
                for hh in range(2):
                    sl = slice(nj * 1024 + hh * 512, nj * 1024 + (hh + 1) * 512)
                    for u in range(CB // 2):
                        nc.tensor.matmul(
                            o2[:, hh * 512 : (hh + 1) * 512],
                            lhsT=attT2[s][u][:, :, i * P : (i + 1) * P],
                            rhs=vb2[s][u][:, :, sl],
                            perf_mode=DR,
                            start=(u == 0),
                            stop=(u == CB // 2 - 1),
                        )
                nc.scalar.activation(
                    er[:, nj * 1024 : (nj + 1) * 1024],
                    o2,
                    AF.Exp,
                    scale=r1s[s][i],
                    accum_out=z2p[:, nj : nj + 1],
                )
                if interleave_ts is not None:
                    v_transposes(
                        1, interleave_ts[nj], interleave_ts[nj] + 1, evict_eng="s"
                    )
            z2 = small.tile([P, 1], f32, tag="sm", name=f"z2_{s}_{i}")
            nc.vector.reduce_sum(z2, z2p, axis=AX.X)
            r2 = small.tile([P, 1], f32, tag="sm", name=f"r2_{s}_{i}")
            nc.vector.reciprocal(r2, z2)
            gz = small.tile([P, 1], f32, tag="sm", name=f"gz_{s}_{i}")
            nc.vector.tensor_scalar_mul(gz, r2, gamma_sb)
            xt = xh[s][i]
            for h in range(4):
                sl = slice(h * (HW // 4), (h + 1) * (HW // 4))
                nc.vector.scalar_tensor_tensor(
                    out=xt[:, sl],
                    in0=er[:, sl],
                    scalar=gz,
                    in1=xt[:, sl],
                    op0=ALU.mult,
                    op1=ALU.add,
                )
                nc.sync.dma_start(
                    out=out[s, i * P : (i + 1) * P, sl],
                    in_=xt[:, sl],
                )

        # ---- software pipeline across the two samples ----
        loads(0, [256, 256, 512, 1024, 1024, 1024])
        loads(1, [2048, 2048])
        vb_alloc(0)
        # sample-0 E quad: rows 0,1 in the E pool; rows 2,3 borrow the mm2
        # chunk slots (idle until sample-0's mm2 phase begins).
        Es[0] = [
            psum_E.tile([P, C], f32, tag="E", name="E_0_0"),
            psum_E.tile([P, C], f32, tag="E", name="E_0_1"),
            psum_o.tile([P, C], f32, tag="o", name="E_0_2"),
            psum_o.tile([P, C], f32, tag="o", name="E_0_3"),
        ]
        for t in range(NT):
            v_transposes(0, t, t + 1)
            for i in range(CB):
                mm1_block(0, i, t)
            if t == 1:
                pe_warm(40)
            elif t >= 2:
                pe_warm(3)
            if t in (3, 7, 11):
                vb_cast_chunk_eng(0, t // 4)
        for i in range(CB):
            softmax1_tail(0, i)
        vb_cast_chunk_eng(0, 3)
        att_transposes(0)
        pe_warm(4)
        vb_alloc(1)
        vb_casts_dma(1)
        # sample-1 rows 0,1 accumulate through sample-0's mm2 phase
        Es[1][0] = psum_E.tile([P, C], f32, tag="E", name="E_1_0")
        Es[1][1] = psum_E.tile([P, C], f32, tag="E", name="E_1_1")
        prev_ts = []
        for i in range(CB):
            for t in prev_ts:
                mm1_block(1, 0, t)
                mm1_block(1, 1, t)
            prev_ts = list(range(i * NJ, (i + 1) * NJ))
            mm2_final(0, i, interleave_ts=prev_ts)
            pe_warm(3)
        for t in prev_ts:
            mm1_block(1, 0, t)
            mm1_block(1, 1, t)
        softmax1_tail(1, 0)
        softmax1_tail(1, 1)
        # sample-1 rows 2,3: one dense pass over the vT tiles
        Es[1][2] = psum_E.tile([P, C], f32, tag="E", name="E_1_2")
        Es[1][3] = psum_E.tile([P, C], f32, tag="E", name="E_1_3")
        for t in range(NT):
            mm1_block(1, 2, t)
            mm1_block(1, 3, t)
        softmax1_tail(1, 2)
        softmax1_tail(1, 3)
        att_transposes(1)
        for i in range(CB):
            mm2_final(1, i)
            pe_warm(2)

    nc.compile()
    return nc


def get_nc():
    global _NC
    if _NC is None:
        _NC = _build_nc()
    return _NC


def kernel(x: np.ndarray, gamma: np.ndarray) -> np.ndarray:
    from concourse.bass_utils import run_bass_kernel_spmd

    B, Cx, H, W = x.shape
    assert (B, Cx, H * W) == (16, C, HW), (B, Cx, H, W)
    nc = get_nc()
    xs = np.ascontiguousarray(np.asarray(x, dtype=np.float32)).reshape(B, Cx, H * W)
    g = np.ascontiguousarray(np.asarray(gamma, dtype=np.float32)).reshape(1)
    in_maps = [{"x": xs[S * c : S * (c + 1)], "gamma": g} for c in range(N_CORES)]
    res = run_bass_kernel_spmd(nc, in_maps, core_ids=list(range(N_CORES)))
    out = np.concatenate(
        [np.asarray(res.results[c]["out"]).astype(np.float32) for c in range(N_CORES)],
        axis=0,
    )
    return out.reshape(B, Cx, H, W)


# revision 14
# speedup vs baseline: 1.1538x; 1.0503x over previous
"""CAM (channel attention) module kernel for Trainium2, 8-core data-parallel.

Reference computation (per sample, C=512, HW=4096):
    v = x.reshape(C, HW)
    E = v @ v.T                                  # (C, C)
    att = softmax(rowmax(E) - E, axis=-1)        # == softmax(-E) stabilized at rowmin
    o = att @ v                                  # (C, HW)
    o = softmax(o, axis=-1)
    out = x + gamma * o
Sharding: data-parallel over batch B=16 -> 2 samples per NeuronCore.

v6 notes:
- x loaded via SWDGE cast-DMA (fp32 HBM -> bf16 SBUF); output stored bf16
  (host upcasts), halving HBM store traffic.  out == bf16(x) for gamma=0.
- PSUM (8 banks): E-pair pool (2) + transpose-pair pool (2) + two
  [P,1024] mm2 chunks (4).  Sample 0's mm1 still runs all 4 row-blocks
  concurrently through the load window by borrowing the then-idle mm2
  chunk slots for E rows 2,3; sample 1 accumulates rows 0,1 during
  sample 0's mm2 phase and rows 2,3 in one dense pass after it.
- Sample 1's transposes are interleaved between mm2 PSUM chunks so the
  PE fills its exp-drain waits; its mm1 blocks are emitted one iteration
  later so eviction latency never stalls the PE queue.
- fp8 natural-layout v: sample 0 via ACT/DVE column-chunk copies inside
  the load window, sample 1 via SWDGE cast-DMAs on post-load idle DMA.
- Softmax1 normalization is folded into the second exp's scale; final
  out = x + (gamma/Z2)*exp runs as half-width DVE chunks, each followed
  by its bf16 half-store.
"""

import sys

if "/opt/trn_rl_repo" not in sys.path:
    sys.path.insert(0, "/opt/trn_rl_repo")

from contextlib import ExitStack

import numpy as np

P = 128
C = 512
HW = 4096
HH = HW // 2
S = 2  # samples per core
CB = C // P  # 4 channel blocks
NB = HW // P  # 32 spatial blocks
NT = NB // 2  # 16 DoubleRow k-pairs for matmul 1
NJ = HW // 1024  # 4 psum chunks for the second matmul
N_CORES = 8

_NC = None


def _build_nc():
    import concourse.bacc as bacc
    import concourse.mybir as mybir
    import concourse.tile as tile
    from concourse.masks import make_identity

    f32 = mybir.dt.float32
    bf16 = mybir.dt.bfloat16
    fp8 = mybir.dt.float8e4
    AF = mybir.ActivationFunctionType
    ALU = mybir.AluOpType
    AX = mybir.AxisListType
    DR = mybir.MatmulPerfMode.DoubleRow

    nc = bacc.Bacc(
        "TRN2",
        target_bir_lowering=False,
        debug=False,
        num_devices=N_CORES,
        num_swdge_queues=4,
    )
    x = nc.dram_tensor("x", (S, C, HW), f32, kind="ExternalInput").ap()
    gamma = nc.dram_tensor("gamma", (1,), f32, kind="ExternalInput").ap()
    out = nc.dram_tensor("out", (S, C, HW), bf16, kind="ExternalOutput").ap()

    with tile.TileContext(nc) as tc, ExitStack() as ctx:
        const = ctx.enter_context(tc.tile_pool(name="const", bufs=1))
        identb = const.tile([P, P], bf16)
        make_identity(nc, identb)
        gamma_sb = const.tile([P, 1], f32)
        nc.sync.dma_start(out=gamma_sb, in_=gamma.to_broadcast((P, 1)))

        xf_pool = ctx.enter_context(tc.tile_pool(name="xf_pool", bufs=8))
        vb_pool = ctx.enter_context(tc.tile_pool(name="vb_pool", bufs=3))
        vt_pool = ctx.enter_context(tc.tile_pool(name="vt_pool", bufs=NT + 2))
        att_pool = ctx.enter_context(tc.tile_pool(name="att_pool", bufs=CB + 1))
        attT_pool = ctx.enter_context(tc.tile_pool(name="attT_pool", bufs=3))
        exp_pool = ctx.enter_context(tc.tile_pool(name="exp_pool", bufs=5))
        small = ctx.enter_context(tc.tile_pool(name="small", bufs=24))
        r1_pool = ctx.enter_context(tc.tile_pool(name="r1_pool", bufs=10))
        psum_E = ctx.enter_context(tc.tile_pool(name="psum_E", bufs=2, space="PSUM"))
        psum_pt = ctx.enter_context(tc.tile_pool(name="psum_pt", bufs=2, space="PSUM"))
        psum_o = ctx.enter_context(tc.tile_pool(name="psum_o", bufs=2, space="PSUM"))

        # per-sample state
        xh = [[None] * CB for _ in range(S)]
        vb2 = [[None] * (CB // 2) for _ in range(S)]
        vT2 = [[None] * NT for _ in range(S)]
        att8 = [[None] * CB for _ in range(S)]
        r1s = [[None] * CB for _ in range(S)]
        attT2 = [[None] * (CB // 2) for _ in range(S)]
        Es = [[None] * CB for _ in range(S)]

        def loads(s, chunks):
            # SWDGE cast-DMA loads: fp32 HBM -> bf16 SBUF, column-chunked so
            # the first transposes start as soon as the first chunk lands.
            for i in range(CB):
                xh[s][i] = xf_pool.tile([P, HW], bf16, tag="xf", name=f"xf_{s}_{i}")
            c0 = 0
            for w in chunks:
                for i in range(CB):
                    nc.gpsimd.dma_start(
                        out=xh[s][i][:, c0 : c0 + w],
                        in_=x[s, i * P : (i + 1) * P, c0 : c0 + w],
                    )
                c0 += w
            assert c0 == HW

        def vb_alloc(s):
            for u in range(CB // 2):
                vb2[s][u] = vb_pool.tile(
                    [P, 2, HW], fp8, tag="vb", name=f"vb2_{s}_{u}"
                )

        def vb_cast_chunk_eng(s, q):
            # bf16 -> fp8 natural-layout copies, one 1024-col chunk, ACT/DVE.
            sl = slice(q * 1024, (q + 1) * 1024)
            for u in range(CB // 2):
                for ko in range(2):
                    i = 2 * u + ko
                    if i % 2 == 0:
                        nc.scalar.copy(vb2[s][u][:, ko, sl], xh[s][i][:, sl])
                    else:
                        nc.vector.tensor_copy(vb2[s][u][:, ko, sl], xh[s][i][:, sl])

        def vb_casts_dma(s):
            # bf16 -> fp8 SWDGE cast-DMAs, half-width; drain post-load on
            # otherwise-idle DMA engines.
            for h in range(2):
                sl = slice(h * HH, (h + 1) * HH)
                for u in range(CB // 2):
                    for ko in range(2):
                        i = 2 * u + ko
                        nc.gpsimd.dma_start(
                            out=vb2[s][u][:, ko, sl], in_=xh[s][i][:, sl]
                        )

        def v_transposes(s, ts, te, evict_eng=None):
            # vT pairs (n-part, c-free) fp8: bf16 PE transposes (PSUM dtype
            # matches input), cast to fp8 during the PSUM->SBUF eviction.
            for t in range(ts, te):
                vt_ = vt_pool.tile([P, 2, C], fp8, tag="vt", name=f"vT2_{s}_{t}")
                for ko in range(2):
                    k = 2 * t + ko
                    pt = psum_pt.tile([P, C], bf16, tag="pt", name=f"ptv_{s}_{k}")
                    for i in range(CB):
                        nc.tensor.transpose(
                            pt[:, i * P : (i + 1) * P],
                            xh[s][i][:, k * P : (k + 1) * P],
                            identb,
                        )
                    eng = evict_eng if evict_eng else ("v" if k % 2 else "s")
                    if eng == "v":
                        nc.vector.tensor_copy(vt_[:, ko, :], pt)
                    else:
                        nc.scalar.copy(vt_[:, ko, :], pt)
                vT2[s][t] = vt_

        def softmax1_tail(s, i):
            E = Es[s][i]
            m = small.tile([P, 1], f32, tag="sm", name=f"m_{s}_{i}")
            nc.vector.tensor_reduce(m, E, axis=AX.X, op=ALU.min)
            a = att_pool.tile([P, C], bf16, tag="att", name=f"att_{s}_{i}")
            z1 = small.tile([P, 1], f32, tag="sm", name=f"z1_{s}_{i}")
            nc.scalar.activation(a, E, AF.Exp, bias=m, scale=-1.0, accum_out=z1)
            r1 = r1_pool.tile([P, 1], f32, tag="r1", name=f"r1_{s}_{i}")
            nc.vector.reciprocal(r1, z1)
            att8[s][i] = a
            r1s[s][i] = r1

        def pe_warm(n):
            # dependency-free LDWEIGHTS keep the PE HAM activity monitor
            # from re-throttling the clock across data-wait gaps; they run
            # the instant the PE reaches them and are overwritten by the
            # next matmul's own weight load.
            for _ in range(n):
                nc.tensor.ldweights(identb)

        def mm1_block(s, i, t):
            nc.tensor.matmul(
                Es[s][i],
                lhsT=vT2[s][t][:, :, i * P : (i + 1) * P],
                rhs=vT2[s][t],
                perf_mode=DR,
                start=(t == 0),
                stop=(t == NT - 1),
            )

        def att_transposes(s):
            # attT pairs (col-part, row-free) fp8 via bf16 PE transpose
            for u in range(CB // 2):
                st = attT_pool.tile([P, 2, C], fp8, tag="attT", name=f"attT2_{s}_{u}")
                for ko in range(2):
                    j = 2 * u + ko
                    pt = psum_pt.tile([P, C], bf16, tag="pt", name=f"pta_{s}_{j}")
                    for i in range(CB):
                        nc.tensor.transpose(
                            pt[:, i * P : (i + 1) * P],
                            att8[s][i][:, j * P : (j + 1) * P],
                            identb,
                        )
                    if j % 2 == 0 and s == 0:
                        nc.vector.tensor_copy(st[:, ko, :], pt)
                    else:
                        nc.scalar.copy(st[:, ko, :], pt)
                attT2[s][u] = st

        ers = [[None] * CB for _ in range(S)]
        z2ps = [[None] * CB for _ in range(S)]

        def mm2_final(s, i, interleave_ts=None, finish=True):
            # o = att @ v (DoubleRow) in [P,1024] PSUM chunks, softmax over
            # HW (1/Z1 folded into the exp scale).  interleave_ts: sample-1
            # transpose steps slotted between chunks so the PE rides out the
            # exp drain of each chunk.  Then out = x + (gamma/Z2)*exp in
            # half-width DVE chunks, each followed by its bf16 half-store.
            er = exp_pool.tile([P, HW], bf16, tag="er", name=f"er_{s}_{i}")
            z2p = small.tile([P, NJ], f32, tag="z2p", name=f"z2p_{s}_{i}")
            for nj in range(NJ):
                o2 = psum_o.tile([P, 1024], f32, tag="o", name=f"o2_{s}_{i}_{nj}")
                for hh in range(2):
                    sl = slice(nj * 1024 + hh * 512, nj * 1024 + (hh + 1) * 512)
                    for u in range(CB // 2):
                        nc.tensor.matmul(
                            o2[:, hh * 512 : (hh + 1) * 512],
                            lhsT=attT2[s][u][:, :, i * P : (i + 1) * P],
                            rhs=vb2[s][u][:, :, sl],
                            perf_mode=DR,
                            start=(u == 0),
                            stop=(u == CB // 2 - 1),
                        )
                nc.scalar.activation(
                    er[:, nj * 1024 : (nj + 1) * 1024],
                    o2,
                    AF.Exp,
                    scale=r1s[s][i],
                    accum_out=z2p[:, nj : nj + 1],
                )
                if interleave_ts is not None:
                    v_transposes(
                        1, interleave_ts[nj], interleave_ts[nj] + 1, evict_eng="v"
                    )
            ers[s][i] = er
            z2ps[s][i] = z2p
            if finish:
                mm2_finish(s, i)

        def mm2_finish(s, i):
            er = ers[s][i]
            z2p = z2ps[s][i]
            z2 = small.tile([P, 1], f32, tag="sm", name=f"z2_{s}_{i}")
            nc.vector.reduce_sum(z2, z2p, axis=AX.X)
            r2 = small.tile([P, 1], f32, tag="sm", name=f"r2_{s}_{i}")
            nc.vector.reciprocal(r2, z2)
            gz = small.tile([P, 1], f32, tag="sm", name=f"gz_{s}_{i}")
            nc.vector.tensor_scalar_mul(gz, r2, gamma_sb)
            xt = xh[s][i]
            for h in range(4):
                sl = slice(h * (HW // 4), (h + 1) * (HW // 4))
                nc.vector.scalar_tensor_tensor(
                    out=xt[:, sl],
                    in0=er[:, sl],
                    scalar=gz,
                    in1=xt[:, sl],
                    op0=ALU.mult,
                    op1=ALU.add,
                )
                nc.sync.dma_start(
                    out=out[s, i * P : (i + 1) * P, sl],
                    in_=xt[:, sl],
                )

        # ---- software pipeline across the two samples ----
        loads(0, [256, 256, 512, 1024, 1024, 1024])
        loads(1, [2048, 2048])
        vb_alloc(0)
        # sample-0 E quad: rows 0,1 in the E pool; rows 2,3 borrow the mm2
        # chunk slots (idle until sample-0's mm2 phase begins).
        Es[0] = [
            psum_E.tile([P, C], f32, tag="E", name="E_0_0"),
            psum_E.tile([P, C], f32, tag="E", name="E_0_1"),
            psum_o.tile([P, C], f32, tag="o", name="E_0_2"),
            psum_o.tile([P, C], f32, tag="o", name="E_0_3"),
        ]
        for t in range(NT):
            v_transposes(0, t, t + 1)
            for i in range(CB):
                mm1_block(0, i, t)
            if t == 1:
                pe_warm(40)
            elif t >= 2:
                pe_warm(3)
            if t in (3, 7, 11):
                vb_cast_chunk_eng(0, t // 4)
        for i in range(CB):
            softmax1_tail(0, i)
        vb_cast_chunk_eng(0, 3)
        att_transposes(0)
        pe_warm(4)
        vb_alloc(1)
        vb_casts_dma(1)
        # sample-1 rows 0,1 accumulate through sample-0's mm2 phase
        Es[1][0] = psum_E.tile([P, C], f32, tag="E", name="E_1_0")
        Es[1][1] = psum_E.tile([P, C], f32, tag="E", name="E_1_1")
        prev_ts = []
        for i in range(CB):
            for t in prev_ts:
                mm1_block(1, 0, t)
                mm1_block(1, 1, t)
            prev_ts = list(range(i * NJ, (i + 1) * NJ))
            mm2_final(0, i, interleave_ts=prev_ts, finish=False)
            pe_warm(3)
        for t in prev_ts:
            mm1_block(1, 0, t)
            mm1_block(1, 1, t)
        softmax1_tail(1, 0)
        softmax1_tail(1, 1)
        mm2_finish(0, 0)
        mm2_finish(0, 1)
        # sample-1 rows 2,3: one dense pass over the vT tiles
        Es[1][2] = psum_E.tile([P, C], f32, tag="E", name="E_1_2")
        Es[1][3] = psum_E.tile([P, C], f32, tag="E", name="E_1_3")
        for t in range(NT):
            mm1_block(1, 2, t)
            mm1_block(1, 3, t)
        mm2_finish(0, 2)
        softmax1_tail(1, 2)
        softmax1_tail(1, 3)
        att_transposes(1)
        mm2_finish(0, 3)
        for i in range(CB):
            mm2_final(1, i)
            pe_warm(2)

    nc.compile()
    return nc


def get_nc():
    global _NC
    if _NC is None:
        _NC = _build_nc()
    return _NC


def kernel(x: np.ndarray, gamma: np.ndarray) -> np.ndarray:
    from concourse.bass_utils import run_bass_kernel_spmd

    B, Cx, H, W = x.shape
    assert (B, Cx, H * W) == (16, C, HW), (B, Cx, H, W)
    nc = get_nc()
    xs = np.ascontiguousarray(np.asarray(x, dtype=np.float32)).reshape(B, Cx, H * W)
    g = np.ascontiguousarray(np.asarray(gamma, dtype=np.float32)).reshape(1)
    in_maps = [{"x": xs[S * c : S * (c + 1)], "gamma": g} for c in range(N_CORES)]
    res = run_bass_kernel_spmd(nc, in_maps, core_ids=list(range(N_CORES)))
    out = np.concatenate(
        [np.asarray(res.results[c]["out"]).astype(np.float32) for c in range(N_CORES)],
        axis=0,
    )
    return out.reshape(B, Cx, H, W)


# revision 15
# speedup vs baseline: 1.1938x; 1.0347x over previous
"""CAM (channel attention) module kernel for Trainium2, 8-core data-parallel.

Reference computation (per sample, C=512, HW=4096):
    v = x.reshape(C, HW)
    E = v @ v.T                                  # (C, C)
    att = softmax(rowmax(E) - E, axis=-1)        # == softmax(-E) stabilized at rowmin
    o = att @ v                                  # (C, HW)
    o = softmax(o, axis=-1)
    out = x + gamma * o
Sharding: data-parallel over batch B=16 -> 2 samples per NeuronCore.

v6 notes:
- x loaded via SWDGE cast-DMA (fp32 HBM -> bf16 SBUF); output stored bf16
  (host upcasts), halving HBM store traffic.  out == bf16(x) for gamma=0.
- PSUM (8 banks): E-pair pool (2) + transpose-pair pool (2) + two
  [P,1024] mm2 chunks (4).  Sample 0's mm1 still runs all 4 row-blocks
  concurrently through the load window by borrowing the then-idle mm2
  chunk slots for E rows 2,3; sample 1 accumulates rows 0,1 during
  sample 0's mm2 phase and rows 2,3 in one dense pass after it.
- Sample 1's transposes are interleaved between mm2 PSUM chunks so the
  PE fills its exp-drain waits; its mm1 blocks are emitted one iteration
  later so eviction latency never stalls the PE queue.
- fp8 natural-layout v: sample 0 via ACT/DVE column-chunk copies inside
  the load window, sample 1 via SWDGE cast-DMAs on post-load idle DMA.
- Softmax1 normalization is folded into the second exp's scale; final
  out = x + (gamma/Z2)*exp runs as half-width DVE chunks, each followed
  by its bf16 half-store.
"""

import sys

if "/opt/trn_rl_repo" not in sys.path:
    sys.path.insert(0, "/opt/trn_rl_repo")

from contextlib import ExitStack

import numpy as np

P = 128
C = 512
HW = 4096
HH = HW // 2
S = 2  # samples per core
CB = C // P  # 4 channel blocks
NB = HW // P  # 32 spatial blocks
NT = NB // 2  # 16 DoubleRow k-pairs for matmul 1
NJ = HW // 1024  # 4 psum chunks for the second matmul
N_CORES = 8

_NC = None


def _build_nc():
    import concourse.bacc as bacc
    import concourse.mybir as mybir
    import concourse.tile as tile
    from concourse.masks import make_identity

    f32 = mybir.dt.float32
    bf16 = mybir.dt.bfloat16
    fp8 = mybir.dt.float8e4
    AF = mybir.ActivationFunctionType
    ALU = mybir.AluOpType
    AX = mybir.AxisListType
    DR = mybir.MatmulPerfMode.DoubleRow

    nc = bacc.Bacc(
        "TRN2",
        target_bir_lowering=False,
        debug=False,
        num_devices=N_CORES,
        num_swdge_queues=4,
    )
    x = nc.dram_tensor("x", (S, C, HW), f32, kind="ExternalInput").ap()
    gamma = nc.dram_tensor("gamma", (1,), f32, kind="ExternalInput").ap()
    out = nc.dram_tensor("out", (S, C, HW), bf16, kind="ExternalOutput").ap()

    with tile.TileContext(nc) as tc, ExitStack() as ctx:
        const = ctx.enter_context(tc.tile_pool(name="const", bufs=1))
        identb = const.tile([P, P], bf16)
        make_identity(nc, identb)
        gamma_sb = const.tile([P, 1], f32)
        nc.sync.dma_start(out=gamma_sb, in_=gamma.to_broadcast((P, 1)))

        xf_pool = ctx.enter_context(tc.tile_pool(name="xf_pool", bufs=8))
        vb_pool = ctx.enter_context(tc.tile_pool(name="vb_pool", bufs=3))
        vt_pool = ctx.enter_context(tc.tile_pool(name="vt_pool", bufs=NT + 2))
        att_pool = ctx.enter_context(tc.tile_pool(name="att_pool", bufs=CB + 1))
        attT_pool = ctx.enter_context(tc.tile_pool(name="attT_pool", bufs=3))
        exp_pool = ctx.enter_context(tc.tile_pool(name="exp_pool", bufs=5))
        small = ctx.enter_context(tc.tile_pool(name="small", bufs=24))
        r1_pool = ctx.enter_context(tc.tile_pool(name="r1_pool", bufs=10))
        psum_E = ctx.enter_context(tc.tile_pool(name="psum_E", bufs=2, space="PSUM"))
        psum_pt = ctx.enter_context(tc.tile_pool(name="psum_pt", bufs=2, space="PSUM"))
        psum_o = ctx.enter_context(tc.tile_pool(name="psum_o", bufs=2, space="PSUM"))

        # per-sample state
        xh = [[None] * CB for _ in range(S)]
        vb2 = [[None] * (CB // 2) for _ in range(S)]
        vT2 = [[None] * NT for _ in range(S)]
        att8 = [[None] * CB for _ in range(S)]
        r1s = [[None] * CB for _ in range(S)]
        attT2 = [[None] * (CB // 2) for _ in range(S)]
        Es = [[None] * CB for _ in range(S)]

        def loads(s, chunks):
            # SWDGE cast-DMA loads: fp32 HBM -> bf16 SBUF, column-chunked so
            # the first transposes start as soon as the first chunk lands.
            for i in range(CB):
                xh[s][i] = xf_pool.tile([P, HW], bf16, tag="xf", name=f"xf_{s}_{i}")
            c0 = 0
            for w in chunks:
                for i in range(CB):
                    nc.gpsimd.dma_start(
                        out=xh[s][i][:, c0 : c0 + w],
                        in_=x[s, i * P : (i + 1) * P, c0 : c0 + w],
                    )
                c0 += w
            assert c0 == HW

        def vb_alloc(s):
            for u in range(CB // 2):
                vb2[s][u] = vb_pool.tile(
                    [P, 2, HW], fp8, tag="vb", name=f"vb2_{s}_{u}"
                )

        def vb_cast_chunk_eng(s, q):
            # bf16 -> fp8 natural-layout copies, one 1024-col chunk, ACT/DVE.
            sl = slice(q * 1024, (q + 1) * 1024)
            for u in range(CB // 2):
                for ko in range(2):
                    i = 2 * u + ko
                    if i % 2 == 0:
                        nc.scalar.copy(vb2[s][u][:, ko, sl], xh[s][i][:, sl])
                    else:
                        nc.vector.tensor_copy(vb2[s][u][:, ko, sl], xh[s][i][:, sl])

        def vb_casts_dma(s):
            # bf16 -> fp8 SWDGE cast-DMAs, half-width; drain post-load on
            # otherwise-idle DMA engines.
            for h in range(2):
                sl = slice(h * HH, (h + 1) * HH)
                for u in range(CB // 2):
                    for ko in range(2):
                        i = 2 * u + ko
                        nc.gpsimd.dma_start(
                            out=vb2[s][u][:, ko, sl], in_=xh[s][i][:, sl]
                        )

        def v_transposes(s, ts, te, evict_eng=None):
            # vT pairs (n-part, c-free) fp8: bf16 PE transposes (PSUM dtype
            # matches input), cast to fp8 during the PSUM->SBUF eviction.
            for t in range(ts, te):
                vt_ = vt_pool.tile([P, 2, C], fp8, tag="vt", name=f"vT2_{s}_{t}")
                for ko in range(2):
                    k = 2 * t + ko
                    pt = psum_pt.tile([P, C], bf16, tag="pt", name=f"ptv_{s}_{k}")
                    for i in range(CB):
                        nc.tensor.transpose(
                            pt[:, i * P : (i + 1) * P],
                            xh[s][i][:, k * P : (k + 1) * P],
                            identb,
                        )
                    eng = evict_eng if evict_eng else ("v" if k % 2 else "s")
                    if eng == "v":
                        nc.vector.tensor_copy(vt_[:, ko, :], pt)
                    else:
                        nc.scalar.copy(vt_[:, ko, :], pt)
                vT2[s][t] = vt_

        def softmax1_tail(s, i):
            E = Es[s][i]
            m = small.tile([P, 1], f32, tag="sm", name=f"m_{s}_{i}")
            nc.vector.tensor_reduce(m, E, axis=AX.X, op=ALU.min)
            a = att_pool.tile([P, C], bf16, tag="att", name=f"att_{s}_{i}")
            z1 = small.tile([P, 1], f32, tag="sm", name=f"z1_{s}_{i}")
            nc.scalar.activation(a, E, AF.Exp, bias=m, scale=-1.0, accum_out=z1)
            r1 = r1_pool.tile([P, 1], f32, tag="r1", name=f"r1_{s}_{i}")
            nc.vector.reciprocal(r1, z1)
            att8[s][i] = a
            r1s[s][i] = r1

        def pe_warm(n):
            # dependency-free LDWEIGHTS keep the PE HAM activity monitor
            # from re-throttling the clock across data-wait gaps; they run
            # the instant the PE reaches them and are overwritten by the
            # next matmul's own weight load.
            for _ in range(n):
                nc.tensor.ldweights(identb)

        def mm1_block(s, i, t):
            nc.tensor.matmul(
                Es[s][i],
                lhsT=vT2[s][t][:, :, i * P : (i + 1) * P],
                rhs=vT2[s][t],
                perf_mode=DR,
                start=(t == 0),
                stop=(t == NT - 1),
            )

        def att_transposes(s):
            # attT pairs (col-part, row-free) fp8 via bf16 PE transpose
            for u in range(CB // 2):
                st = attT_pool.tile([P, 2, C], fp8, tag="attT", name=f"attT2_{s}_{u}")
                for ko in range(2):
                    j = 2 * u + ko
                    pt = psum_pt.tile([P, C], bf16, tag="pt", name=f"pta_{s}_{j}")
                    for i in range(CB):
                        nc.tensor.transpose(
                            pt[:, i * P : (i + 1) * P],
                            att8[s][i][:, j * P : (j + 1) * P],
                            identb,
                        )
                    if j % 2 == 0 and s == 0:
                        nc.vector.tensor_copy(st[:, ko, :], pt)
                    else:
                        nc.scalar.copy(st[:, ko, :], pt)
                attT2[s][u] = st

        ers = [[None] * CB for _ in range(S)]
        z2ps = [[None] * CB for _ in range(S)]

        def mm2_final(s, i, interleave_ts=None, finish=True):
            # o = att @ v (DoubleRow) in [P,1024] PSUM chunks, softmax over
            # HW (1/Z1 folded into the exp scale).  interleave_ts: sample-1
            # transpose steps slotted between chunks so the PE rides out the
            # exp drain of each chunk.  Then out = x + (gamma/Z2)*exp in
            # half-width DVE chunks, each followed by its bf16 half-store.
            er = exp_pool.tile([P, HW], bf16, tag="er", name=f"er_{s}_{i}")
            z2p = small.tile([P, NJ], f32, tag="z2p", name=f"z2p_{s}_{i}")
            for nj in range(NJ):
                o2 = psum_o.tile([P, 1024], f32, tag="o", name=f"o2_{s}_{i}_{nj}")
                for hh in range(2):
                    sl = slice(nj * 1024 + hh * 512, nj * 1024 + (hh + 1) * 512)
                    for u in range(CB // 2):
                        nc.tensor.matmul(
                            o2[:, hh * 512 : (hh + 1) * 512],
                            lhsT=attT2[s][u][:, :, i * P : (i + 1) * P],
                            rhs=vb2[s][u][:, :, sl],
                            perf_mode=DR,
                            start=(u == 0),
                            stop=(u == CB // 2 - 1),
                        )
                nc.scalar.activation(
                    er[:, nj * 1024 : (nj + 1) * 1024],
                    o2,
                    AF.Exp,
                    scale=r1s[s][i],
                    accum_out=z2p[:, nj : nj + 1],
                )
                if interleave_ts is not None:
                    v_transposes(
                        1, interleave_ts[nj], interleave_ts[nj] + 1, evict_eng="v"
                    )
            ers[s][i] = er
            z2ps[s][i] = z2p
            if finish:
                mm2_finish(s, i)

        def mm2_finish(s, i):
            er = ers[s][i]
            z2p = z2ps[s][i]
            z2 = small.tile([P, 1], f32, tag="sm", name=f"z2_{s}_{i}")
            nc.vector.reduce_sum(z2, z2p, axis=AX.X)
            r2 = small.tile([P, 1], f32, tag="sm", name=f"r2_{s}_{i}")
            nc.vector.reciprocal(r2, z2)
            gz = small.tile([P, 1], f32, tag="sm", name=f"gz_{s}_{i}")
            nc.vector.tensor_scalar_mul(gz, r2, gamma_sb)
            xt = xh[s][i]
            for h in range(4):
                sl = slice(h * (HW // 4), (h + 1) * (HW // 4))
                nc.vector.scalar_tensor_tensor(
                    out=xt[:, sl],
                    in0=er[:, sl],
                    scalar=gz,
                    in1=xt[:, sl],
                    op0=ALU.mult,
                    op1=ALU.add,
                )
                nc.sync.dma_start(
                    out=out[s, i * P : (i + 1) * P, sl],
                    in_=xt[:, sl],
                )

        # ---- software pipeline across the two samples ----
        loads(0, [256, 256, 512, 1024, 1024, 1024])
        loads(1, [2048, 2048])
        vb_alloc(0)
        # sample-0 E quad: rows 0,1 in the E pool; rows 2,3 borrow the mm2
        # chunk slots (idle until sample-0's mm2 phase begins).
        Es[0] = [
            psum_E.tile([P, C], f32, tag="E", name="E_0_0"),
            psum_E.tile([P, C], f32, tag="E", name="E_0_1"),
            psum_o.tile([P, C], f32, tag="o", name="E_0_2"),
            psum_o.tile([P, C], f32, tag="o", name="E_0_3"),
        ]
        for t in range(NT):
            v_transposes(0, t, t + 1)
            for i in range(CB):
                mm1_block(0, i, t)
            if t == 1:
                pe_warm(40)
            elif t >= 2:
                pe_warm(3)
            if t in (3, 7, 11):
                vb_cast_chunk_eng(0, t // 4)
        for i in range(CB):
            softmax1_tail(0, i)
        vb_cast_chunk_eng(0, 3)
        att_transposes(0)
        pe_warm(4)
        vb_alloc(1)
        vb_casts_dma(1)
        # sample-1 rows 0,1 accumulate through sample-0's mm2 phase
        Es[1][0] = psum_E.tile([P, C], f32, tag="E", name="E_1_0")
        Es[1][1] = psum_E.tile([P, C], f32, tag="E", name="E_1_1")
        prev_ts = []
        for i in range(CB):
            for t in prev_ts:
                mm1_block(1, 0, t)
                mm1_block(1, 1, t)
            prev_ts = list(range(i * NJ, (i + 1) * NJ))
            mm2_final(0, i, interleave_ts=prev_ts, finish=False)
            pe_warm(3)
        for t in prev_ts:
            mm1_block(1, 0, t)
            mm1_block(1, 1, t)
        softmax1_tail(1, 0)
        softmax1_tail(1, 1)
        mm2_finish(0, 0)
        mm2_finish(0, 1)
        # sample-1 rows 2,3: one dense pass over the vT tiles
        Es[1][2] = psum_E.tile([P, C], f32, tag="E", name="E_1_2")
        Es[1][3] = psum_E.tile([P, C], f32, tag="E", name="E_1_3")
        for t in range(NT):
            mm1_block(1, 2, t)
            mm1_block(1, 3, t)
        softmax1_tail(1, 2)
        softmax1_tail(1, 3)
        mm2_finish(0, 2)
        att_transposes(1)
        mm2_finish(0, 3)
        for i in range(CB):
            mm2_final(1, i)
            pe_warm(2)

    nc.compile()
    return nc


def get_nc():
    global _NC
    if _NC is None:
        _NC = _build_nc()
    return _NC


def kernel(x: np.ndarray, gamma: np.ndarray) -> np.ndarray:
    from concourse.bass_utils import run_bass_kernel_spmd

    B, Cx, H, W = x.shape
    assert (B, Cx, H * W) == (16, C, HW), (B, Cx, H, W)
    nc = get_nc()
    xs = np.ascontiguousarray(np.asarray(x, dtype=np.float32)).reshape(B, Cx, H * W)
    g = np.ascontiguousarray(np.asarray(gamma, dtype=np.float32)).reshape(1)
    in_maps = [{"x": xs[S * c : S * (c + 1)], "gamma": g} for c in range(N_CORES)]
    res = run_bass_kernel_spmd(nc, in_maps, core_ids=list(range(N_CORES)))
    out = np.concatenate(
        [np.asarray(res.results[c]["out"]).astype(np.float32) for c in range(N_CORES)],
        axis=0,
    )
    return out.reshape(B, Cx, H, W)
